# revision 1
# baseline (speedup 1.0000x reference)
"""Trainium2 Bass kernel for nn_AttentionBlock (B=4, H=W=64, C=256, D=32).

Sharding: 8 shards = 4 samples x 2 query-halves. Each core gets the full
sample's rows (reordered so its 2048 query rows come first), computes K/V
for all 4096 keys, and attention + output projection + residual for its
2048 queries. No collectives needed.

Self-contained: hardcodes shapes, imports only /opt/trn_rl_repo concourse.
"""

import sys

if "/opt/trn_rl_repo" not in sys.path:
    sys.path.insert(0, "/opt/trn_rl_repo")

import numpy as np
import ml_dtypes

BF16 = ml_dtypes.bfloat16

# Problem constants
B, HH, WW, C = 4, 64, 64, 256
D = 32
N = HH * WW          # 4096 keys per sample
NQ = N // 2          # 2048 queries per core
NCORES = 8
KC = N // 128        # 32 key chunks
QB = NQ // 128       # 16 query blocks per core

_compiled_cache = {}


def _build(use_bias: bool):
    from contextlib import ExitStack
    from concourse import bacc, tile, mybir, masks

    f32 = mybir.dt.float32
    bf = mybir.dt.bfloat16

    nc = bacc.Bacc("TRN2", target_bir_lowering=False, debug=False, num_devices=NCORES)

    x16_d = nc.dram_tensor("x16", [N, C], bf, kind="ExternalInput")
    xq32_d = nc.dram_tensor("xq32", [NQ, C], f32, kind="ExternalInput")
    wqa_d = nc.dram_tensor("wqa_rep", [257, 128], bf, kind="ExternalInput")
    wka_d = nc.dram_tensor("wka_rep", [257, 128], bf, kind="ExternalInput")
    wva_d = nc.dram_tensor("wva", [257, 256], bf, kind="ExternalInput")
    woa_d = nc.dram_tensor("woa", [257, 256], bf, kind="ExternalInput")
    out_d = nc.dram_tensor("out", [NQ, C], f32, kind="ExternalOutput")

    Exp = mybir.ActivationFunctionType.Exp
    Add = mybir.AluOpType.add
    Mult = mybir.AluOpType.mult

    with tile.TileContext(nc) as tc:
        with ExitStack() as ctx:
            const = ctx.enter_context(tc.tile_pool(name="const", bufs=1))
            big = ctx.enter_context(tc.tile_pool(name="big", bufs=1))
            expp = ctx.enter_context(tc.tile_pool(name="expp", bufs=6))
            small = ctx.enter_context(tc.tile_pool(name="small", bufs=2))
            ps_s = ctx.enter_context(tc.tile_pool(name="ps_s", bufs=2, space="PSUM"))
            ps_att = ctx.enter_context(tc.tile_pool(name="ps_att", bufs=2, space="PSUM"))
            ps_misc = ctx.enter_context(tc.tile_pool(name="ps_misc", bufs=2, space="PSUM"))

            # ---- constants & weights ----
            ident = const.tile([128, 128], bf, tag="ident")
            masks.make_identity(nc, ident[:])
            ones_row = const.tile([1, 512], bf, tag="ones_row")
            nc.gpsimd.memset(ones_row[:], 1.0)

            wq0 = const.tile([128, 128], bf, tag="wq0")
            wq1 = const.tile([128, 128], bf, tag="wq1")
            wk0 = const.tile([128, 128], bf, tag="wk0")
            wk1 = const.tile([128, 128], bf, tag="wk1")
            wv0 = const.tile([128, 256], bf, tag="wv0")
            wv1 = const.tile([128, 256], bf, tag="wv1")
            wo0 = const.tile([128, 256], bf, tag="wo0")
            wo1 = const.tile([128, 256], bf, tag="wo1")
            nc.sync.dma_start(out=wq0[:], in_=wqa_d[0:128, :])
            nc.sync.dma_start(out=wq1[:], in_=wqa_d[128:256, :])
            nc.sync.dma_start(out=wk0[:], in_=wka_d[0:128, :])
            nc.sync.dma_start(out=wk1[:], in_=wka_d[128:256, :])
            nc.sync.dma_start(out=wv0[:], in_=wva_d[0:128, :])
            nc.sync.dma_start(out=wv1[:], in_=wva_d[128:256, :])
            nc.sync.dma_start(out=wo0[:], in_=woa_d[0:128, :])
            nc.sync.dma_start(out=wo1[:], in_=woa_d[128:256, :])
            if use_bias:
                wqb = const.tile([1, 128], bf, tag="wqb")
                wkb = const.tile([1, 128], bf, tag="wkb")
                wvb = const.tile([1, 256], bf, tag="wvb")
                wob = const.tile([1, 256], bf, tag="wob")
                nc.sync.dma_start(out=wqb[:], in_=wqa_d[256:257, :])
                nc.sync.dma_start(out=wkb[:], in_=wka_d[256:257, :])
                nc.sync.dma_start(out=wvb[:], in_=wva_d[256:257, :])
                nc.sync.dma_start(out=wob[:], in_=woa_d[256:257, :])

            # ---- phase A: x -> xT (channel-major), via identity matmuls ----
            # x arrives as [4096, 256] bf16; load as [128, 32, 256] via 8 large
            # strided DMAs (issue cost is ~0.6us per dma_start, so few + big),
            # split across two issuing engines.
            xbig = big.tile([128, KC, 256], bf, tag="xbig")
            x_r = x16_d[:].rearrange("(t p) c -> p t c", p=128)
            for d in range(32):
                nc.sync.dma_start(out=xbig[:, d : d + 1, :], in_=x_r[:, d : d + 1, :])
            xT = big.tile([128, 2, N], bf, tag="xT")  # [:, h, :]: channels 128h..128h+127
            for t in range(16):
                ta, tb = 2 * t, 2 * t + 1
                pt = ps_s.tile([128, 1024], f32, tag="s")
                nc.tensor.matmul(pt[:, 0:128], xbig[:, ta, 0:128], ident[:], start=True, stop=True)
                nc.tensor.matmul(pt[:, 128:256], xbig[:, tb, 0:128], ident[:], start=True, stop=True)
                nc.tensor.matmul(pt[:, 512:640], xbig[:, ta, 128:256], ident[:], start=True, stop=True)
                nc.tensor.matmul(pt[:, 640:768], xbig[:, tb, 128:256], ident[:], start=True, stop=True)
                nc.vector.tensor_copy(xT[:, 0, 256 * t : 256 * t + 256], pt[:, 0:256])
                nc.vector.tensor_copy(xT[:, 1, 256 * t : 256 * t + 256], pt[:, 512:768])

            # ---- phase B: qT/kT (d on partitions 0..31) and V ----
            qT = big.tile([32, NQ], bf, tag="qT")
            kT = big.tile([32, N], bf, tag="kT")
            for s in range(NQ // 512):
                pq = ps_s.tile([128, 1024], f32, tag="s")
                nc.tensor.matmul(pq[0:32, 0:512], wq0[:, 0:32], xT[:, 0, 512 * s : 512 * s + 512], start=True, stop=False)
                nc.tensor.matmul(pq[0:32, 0:512], wq1[:, 0:32], xT[:, 1, 512 * s : 512 * s + 512], start=False, stop=not use_bias)
                if use_bias:
                    nc.tensor.matmul(pq[0:32, 0:512], wqb[:, 0:32], ones_row[:, 0:512], start=False, stop=True)
                nc.vector.tensor_copy(qT[:, 512 * s : 512 * s + 512], pq[0:32, 0:512])
            for s in range(N // 512):
                pk = ps_s.tile([128, 1024], f32, tag="s")
                nc.tensor.matmul(pk[0:32, 0:512], wk0[:, 0:32], xT[:, 0, 512 * s : 512 * s + 512], start=True, stop=False)
                nc.tensor.matmul(pk[0:32, 0:512], wk1[:, 0:32], xT[:, 1, 512 * s : 512 * s + 512], start=False, stop=not use_bias)
                if use_bias:
                    nc.tensor.matmul(pk[0:32, 0:512], wkb[:, 0:32], ones_row[:, 0:512], start=False, stop=True)
                nc.vector.tensor_copy(kT[:, 512 * s : 512 * s + 512], pk[0:32, 0:512])

            # V rows (keys) with a ones column at 256 for the softmax denominator
            vsb = big.tile([128, KC, 260], bf, tag="vsb")
            nc.vector.memset(vsb[:, :, 256:257], 1.0)
            for m in range(KC):
                pv = ps_misc.tile([128, 256], f32, tag="m")
                nc.tensor.matmul(pv[:], xT[:, 0, 128 * m : 128 * m + 128], wv0[:], start=True, stop=False)
                nc.tensor.matmul(pv[:], xT[:, 1, 128 * m : 128 * m + 128], wv1[:], start=False, stop=not use_bias)
                if use_bias:
                    nc.tensor.matmul(pv[:], ones_row[:, 0:128], wvb[:], start=False, stop=True)
                nc.vector.tensor_copy(vsb[:, m, 0:256], pv[:])

            # ---- phase C: software-pipelined S -> exp -> attend, per 128-query block ----
            def epilogue(qb, pa):
                rec = small.tile([128, 1], f32, tag="rec")
                nc.vector.reciprocal(rec[:], pa[:, 256:257])
                at = small.tile([128, 256], bf, tag="attn")
                nc.vector.tensor_scalar(at[:], pa[:, 0:256], rec[:], None, Mult)
                ptr = ps_misc.tile([128, 256], f32, tag="m")
                nc.tensor.matmul(ptr[:, 0:128], at[:, 0:128], ident[:], start=True, stop=True)
                nc.tensor.matmul(ptr[:, 128:256], at[:, 128:256], ident[:], start=True, stop=True)
                aT = small.tile([128, 256], bf, tag="aT")
                nc.vector.tensor_copy(aT[:], ptr[:])
                po = ps_misc.tile([128, 256], f32, tag="m")
                nc.tensor.matmul(po[:], aT[:, 0:128], wo0[:], start=True, stop=False)
                nc.tensor.matmul(po[:], aT[:, 128:256], wo1[:], start=False, stop=not use_bias)
                if use_bias:
                    nc.tensor.matmul(po[:], ones_row[:, 0:128], wob[:], start=False, stop=True)
                xq = small.tile([128, 256], f32, tag="xq", bufs=3)
                nc.sync.dma_start(out=xq[:], in_=xq32_d[128 * qb : 128 * qb + 128, :])
                ot = small.tile([128, 256], f32, tag="ot", bufs=3)
                nc.vector.tensor_tensor(ot[:], po[:], xq[:], Add)
                nc.sync.dma_start(out=out_d[128 * qb : 128 * qb + 128, :], in_=ot[:])

            # Process query blocks in PAIRS (256 query columns per S matmul):
            # each group g covers qblocks 2g, 2g+1 in 8 steps of 4 key chunks.
            # S psum tile [128, 1024] holds 4 chunks x 256 q; one EXP covers it.
            pa_tiles = {}
            prev = None  # (et, g, t)
            for s in range(8 * (QB // 2) + 1):
                if s < 8 * (QB // 2):
                    g, t = divmod(s, 8)
                    if t == 0:
                        pa_tiles[2 * g] = ps_att.tile([128, 260], f32, tag="a", name=f"pa{2 * g}")
                        pa_tiles[2 * g + 1] = ps_att.tile([128, 260], f32, tag="a", name=f"pa{2 * g + 1}")
                    pst = ps_s.tile([128, 1024], f32, tag="s")
                    for cc in range(4):
                        m = 4 * t + cc
                        nc.tensor.matmul(
                            pst[:, 256 * cc : 256 * cc + 256],
                            kT[:, 128 * m : 128 * m + 128],
                            qT[:, 256 * g : 256 * g + 256],
                            start=True,
                            stop=True,
                        )
                # attend with previous step's exp tile (keeps PE busy during exp)
                if prev is not None:
                    et_p, g_p, t_p = prev
                    for cc in range(4):
                        m = 4 * t_p + cc
                        for h in range(2):
                            nc.tensor.matmul(
                                pa_tiles[2 * g_p + h][:, 0:257],
                                et_p[:, 256 * cc + 128 * h : 256 * cc + 128 * h + 128],
                                vsb[:, m, 0:257],
                                start=(m == 0),
                                stop=(m == KC - 1),
                            )
                    if t_p == 7:
                        for h in range(2):
                            epilogue(2 * g_p + h, pa_tiles[2 * g_p + h])
                            del pa_tiles[2 * g_p + h]
                if s < 8 * (QB // 2):
                    et = expp.tile([128, 1024], bf, tag="e")
                    nc.scalar.activation(et[:], pst[:], Exp)
                    prev = (et, g, t)

    nc.compile()
    return nc


def _get_compiled(use_bias: bool):
    key = bool(use_bias)
    if key not in _compiled_cache:
        _compiled_cache[key] = _build(use_bias)
    return _compiled_cache[key]


def _prep(x, wq, bq, wk, bk, wv, bv, wo, bo):
    xf = np.ascontiguousarray(np.asarray(x, dtype=np.float32)).reshape(B, N, C)
    wq = np.asarray(wq, np.float32)
    bq = np.asarray(bq, np.float32)
    wk = np.asarray(wk, np.float32)
    bk = np.asarray(bk, np.float32)
    wv = np.asarray(wv, np.float32)
    bv = np.asarray(bv, np.float32)
    wo = np.asarray(wo, np.float32)
    bo = np.asarray(bo, np.float32)

    use_bias = not (
        np.all(bq == 0) and np.all(bk == 0) and np.all(bv == 0) and np.all(bo == 0)
    )

    scale = np.float32(1.0 / np.sqrt(np.float32(D)))
    wqa = np.concatenate([wq, bq[None, :]], 0) * scale  # fold softmax scale into q
    wka = np.concatenate([wk, bk[None, :]], 0)
    wqa_rep = np.ascontiguousarray(np.tile(wqa, (1, 4))).astype(BF16)  # [257, 128]
    wka_rep = np.ascontiguousarray(np.tile(wka, (1, 4))).astype(BF16)
    wva = np.concatenate([wv, bv[None, :]], 0).astype(BF16)  # [257, 256]
    woa = np.concatenate([wo, bo[None, :]], 0).astype(BF16)

    in_maps = []
    for core in range(NCORES):
        b, h = divmod(core, 2)
        if h == 0:
            xo = xf[b]
        else:
            xo = np.concatenate([xf[b, NQ:], xf[b, :NQ]], 0)
        in_maps.append(
            {
                "x16": xo.astype(BF16),
                "xq32": np.ascontiguousarray(xo[:NQ]),
                "wqa_rep": wqa_rep,
                "wka_rep": wka_rep,
                "wva": wva,
                "woa": woa,
            }
        )
    return in_maps, use_bias


def _gather(results):
    out = np.empty((B, N, C), np.float32)
    for core in range(NCORES):
        b, h = divmod(core, 2)
        out[b, NQ * h : NQ * (h + 1)] = results[core]["out"]
    return out.reshape(B, HH, WW, C)


def kernel(x, wq, bq, wk, bk, wv, bv, wo, bo):
    from concourse.bass_utils import run_bass_kernel_spmd

    in_maps, use_bias = _prep(x, wq, bq, wk, bk, wv, bv, wo, bo)
    nc = _get_compiled(use_bias)
    res = run_bass_kernel_spmd(nc, in_maps, core_ids=list(range(NCORES)))
    return _gather(res.results)


def _ensure_ntff_hook():
    """The agent image's antenv stub lacks axon_hooks; synthesize it so
    run_bass_kernel_spmd(trace=True) can NTFF-profile via libaxon_pjrt."""
    import types

    try:
        from antenv.axon_hooks import get_axon_ntff_profile_hook  # noqa: F401
        return
    except ImportError:
        pass
    import antenv
    from trn_agent_boot.trn_boot import _ntff_profile_via_ctypes

    mod = types.ModuleType("antenv.axon_hooks")
    state = {"h": _ntff_profile_via_ctypes("/opt/axon/libaxon_pjrt.so")}
    mod.get_axon_ntff_profile_hook = lambda: state["h"]
    mod.set_axon_ntff_profile_hook = lambda h: state.__setitem__("h", h)
    sys.modules["antenv.axon_hooks"] = mod
    antenv.axon_hooks = mod


def run_traced(inputs, **kw):
    """For test.py: run with NTFF profiling; returns (output, BassKernelResults)."""
    from concourse.bass_utils import run_bass_kernel_spmd

    _ensure_ntff_hook()

    in_maps, use_bias = _prep(**inputs)
    nc = _get_compiled(use_bias)
    res = run_bass_kernel_spmd(nc, in_maps, core_ids=list(range(NCORES)), trace=True, **kw)
    return _gather(res.results), res

